# revision 25
# baseline (speedup 1.0000x reference)
"""MoE layer (8 experts, top-2, SwiGLU) for Trainium2, expert-parallel over 8 cores.

Strategy:
  - Router (x @ router_w, top-2, softmax) runs on host in fp32 — it is 0.01%
    of the FLOPs and determines the (data-dependent) sharding.
  - Each core is assigned one expert. Tokens routed to that expert are
    gathered on host, padded to a common capacity C, and shipped transposed
    as xT [D, C] so both GEMMs need no on-device transpose:
        h1T = w1.T @ x.T   (lhsT = w1 [D,Hp], rhs = xT [D,C])   -> [Hp, C]
        h2T = w2.T @ x.T
        hT  = silu(h1T) * h2T
        y   = hT.T @ w3    (lhsT = hT [Hp,C], rhs = w3 [Hp,D])  -> [C, D]
    y rows are scaled by the per-token combine weight on device.
  - Host scatter-adds the 8 per-expert outputs back to [B,S,D].

  Matmuls run in bf16 (fp32 accumulate in PSUM); hidden dim 2730 is padded
  to 2816 = 22*128 (zero pad is exact: silu(0)*0 = 0).

  Token blocks of 512 are processed in PAIRS for mm1/mm2, and mm3 computes
  its two 512-wide output halves in parallel PSUM banks, so each LDWEIGHTS
  feeds two matmuls (measured ~5% faster on HW than one LDW per matmul).
"""

import os

import numpy as np
import ml_dtypes

DIM = 1024
NUM_EXPERTS = 8
HIDDEN = 2730
P = 128
HP = 2816  # hidden padded to 22*128
KD = DIM // P  # 8 contraction chunks for mm1/mm2
HPT = HP // P  # 22 chunks of the hidden dim
NBLK = 512  # token block (moving free dim per matmul)

TRACE = os.environ.get("MOE_TRACE", "0") == "1"
LAST_RESULT = None  # BassKernelResults of the last run (for test harness)

_KERNELS: dict = {}


def _build(C: int, reps: int = 1):
    """Build + compile the per-core Bass kernel for capacity C (multiple of 128).

    reps > 1 wraps the whole body (including DMAs) in a device-side loop that
    recomputes the same result `reps` times — only used for wall-clock
    benchmarking (dispatch overhead cancels in the rep delta).
    """
    import concourse.mybir as mybir
    import concourse.tile as tile
    from concourse import bacc

    dt = mybir.dt
    nc = bacc.Bacc(None, target_bir_lowering=False)

    xt = nc.dram_tensor("xt", [KD, P, C], dt.bfloat16, kind="ExternalInput")
    w1 = nc.dram_tensor("w1", [KD, P, HP], dt.bfloat16, kind="ExternalInput")
    w2 = nc.dram_tensor("w2", [KD, P, HP], dt.bfloat16, kind="ExternalInput")
    w3 = nc.dram_tensor("w3", [HPT, P, DIM], dt.bfloat16, kind="ExternalInput")
    wv = nc.dram_tensor("wv", [P, C // P], dt.float32, kind="ExternalInput")
    y = nc.dram_tensor("y", [C, DIM], dt.float32, kind="ExternalOutput")

    blocks = []
    c0 = 0
    while c0 < C:
        bn = min(NBLK, C - c0)
        blocks.append((c0, bn))
        c0 += bn
    # group full-size blocks into pairs (mm1/mm2 share each LDWEIGHTS
    # between the two blocks of a pair)
    pairs = []
    i = 0
    while i < len(blocks):
        if i + 1 < len(blocks) and blocks[i][1] == NBLK and blocks[i + 1][1] == NBLK:
            pairs.append((blocks[i], blocks[i + 1]))
            i += 2
        else:
            pairs.append((blocks[i],))
            i += 1

    with tile.TileContext(nc) as tc:
        with (
            tc.tile_pool(name="wpool", bufs=1) as wpool,
            tc.tile_pool(name="xpool", bufs=1) as xpool,
            tc.tile_pool(name="hpool", bufs=1) as hpool,
            tc.tile_pool(name="tpool", bufs=2) as tpool,
            tc.tile_pool(name="ypool", bufs=3) as ypool,
            tc.tile_pool(name="psA", bufs=1, space="PSUM") as psA,
            tc.tile_pool(name="psB", bufs=1, space="PSUM") as psB,
            tc.tile_pool(name="psC", bufs=2, space="PSUM") as psC,
        ):

            def make_x(pi, bj, c0, bn):
                t = xpool.tile(
                    [P, KD, NBLK], dt.bfloat16, name=f"x_{pi}_{bj}", tag=f"x_{bj}"
                )
                for kd in range(KD):
                    nc.sync.dma_start(t[:, kd, :bn], xt[kd][:, c0 : c0 + bn])
                return t

            def emit_body():
                # First pair's activations, so mm1 can start early.
                x0 = [
                    make_x(0, bj, c0, bn) for bj, (c0, bn) in enumerate(pairs[0])
                ]

                # Resident weights, DMA'd in hp-sliced parts in the order the
                # first pair's matmuls consume them, split across the SP HWDGE
                # queue (x, w1, w3) and the gpsimd SWDGE queue (w2, wv).
                w1_sb = [
                    wpool.tile([P, HP], dt.bfloat16, name=f"w1_{kd}", tag=f"w1_{kd}")
                    for kd in range(KD)
                ]
                w2_sb = [
                    wpool.tile([P, HP], dt.bfloat16, name=f"w2_{kd}", tag=f"w2_{kd}")
                    for kd in range(KD)
                ]
                w3_sb = [
                    wpool.tile([P, DIM], dt.bfloat16, name=f"w3_{hp}", tag=f"w3_{hp}")
                    for hp in range(HPT)
                ]
                bounds = [0, 3 * P, 7 * P, 12 * P, 17 * P, HP]
                for pi in range(len(bounds) - 1):
                    sl = slice(bounds[pi], bounds[pi + 1])
                    for kd in range(KD):
                        nc.sync.dma_start(w1_sb[kd][:, sl], w1[kd][:, sl])
                    for kd in range(KD):
                        nc.gpsimd.dma_start(w2_sb[kd][:, sl], w2[kd][:, sl])
                for hp in range(HPT):
                    nc.sync.dma_start(w3_sb[hp][:], w3[hp])

                wv_sb = wpool.tile([P, C // P], dt.float32, name="wv_sb", tag="wv_sb")
                nc.gpsimd.dma_start(wv_sb[:], wv[:])

                def mm3_block(pi, bj, c0, bn, h_sb):
                    # y[block] = (hT.T @ w3) * combine_weight; the two 512-wide
                    # output halves accumulate in parallel banks sharing LDW.
                    for cs in range(bn // P):
                        ci = c0 // P + cs
                        ps3 = [
                            psC.tile(
                                [P, 512],
                                dt.float32,
                                name=f"ps3_{pi}_{bj}_{cs}_{dti}",
                                tag=f"ps3_{dti}",
                            )
                            for dti in range(DIM // 512)
                        ]
                        for hp in range(HPT):
                            lhsT = h_sb[hp][:, cs * P : (cs + 1) * P]
                            for dti in range(DIM // 512):
                                nc.tensor.matmul(
                                    ps3[dti][:],
                                    lhsT,
                                    w3_sb[hp][:, dti * 512 : (dti + 1) * 512],
                                    start=(hp == 0),
                                    stop=(hp == HPT - 1),
                                )
                        for dti in range(DIM // 512):
                            yt = ypool.tile(
                                [P, 512],
                                dt.float32,
                                name=f"y_{pi}_{bj}_{cs}_{dti}",
                                tag="yt",
                            )
                            nc.vector.tensor_scalar_mul(
                                yt[:], ps3[dti][:], wv_sb[:, ci : ci + 1]
                            )
                            nc.sync.dma_start(
                                y[
                                    c0 + cs * P : c0 + (cs + 1) * P,
                                    dti * 512 : (dti + 1) * 512,
                                ],
                                yt[:],
                            )

                for pi, pblocks in enumerate(pairs):
                    nb = len(pblocks)
                    if pi == 0:
                        xb = x0
                    else:
                        xb = [
                            make_x(pi, bj, c0, bn)
                            for bj, (c0, bn) in enumerate(pblocks)
                        ]

                    # hT = silu(w1.T @ xT) * (w2.T @ xT) for each block of the
                    # pair; both blocks' matmuls share each LDWEIGHTS.
                    h_sbs = [[] for _ in range(nb)]
                    for hp in range(HPT):
                        ps1 = [
                            psA.tile(
                                [P, pblocks[bj][1]],
                                dt.float32,
                                name=f"ps1_{pi}_{bj}_{hp}",
                                tag=f"ps1_{bj}",
                            )
                            for bj in range(nb)
                        ]
                        for kd in range(KD):
                            lhsT = w1_sb[kd][:, hp * P : (hp + 1) * P]
                            for bj in range(nb):
                                nc.tensor.matmul(
                                    ps1[bj][:],
                                    lhsT,
                                    xb[bj][:, kd, 0 : pblocks[bj][1]],
                                    start=(kd == 0),
                                    stop=(kd == KD - 1),
                                )
                        tsil = []
                        for bj in range(nb):
                            t = tpool.tile(
                                [P, pblocks[bj][1]],
                                dt.bfloat16,
                                name=f"sil_{pi}_{bj}_{hp}",
                                tag=f"sil_{bj}",
                            )
                            nc.scalar.activation(
                                t[:], ps1[bj][:], mybir.ActivationFunctionType.Silu
                            )
                            tsil.append(t)
                        ps2 = [
                            psB.tile(
                                [P, pblocks[bj][1]],
                                dt.float32,
                                name=f"ps2_{pi}_{bj}_{hp}",
                                tag=f"ps2_{bj}",
                            )
                            for bj in range(nb)
                        ]
                        for kd in range(KD):
                            lhsT = w2_sb[kd][:, hp * P : (hp + 1) * P]
                            for bj in range(nb):
                                nc.tensor.matmul(
                                    ps2[bj][:],
                                    lhsT,
                                    xb[bj][:, kd, 0 : pblocks[bj][1]],
                                    start=(kd == 0),
                                    stop=(kd == KD - 1),
                                )
                        for bj in range(nb):
                            ht = hpool.tile(
                                [P, pblocks[bj][1]],
                                dt.bfloat16,
                                name=f"h_{pi}_{bj}_{hp}",
                                tag=f"h_{hp}_{bj}",
                            )
                            nc.vector.tensor_mul(ht[:], tsil[bj][:], ps2[bj][:])
                            h_sbs[bj].append(ht)

                    for bj, (c0, bn) in enumerate(pblocks):
                        mm3_block(pi, bj, c0, bn, h_sbs[bj])

            if reps > 1:
                with tc.For_i(0, reps, 1):
                    emit_body()
            else:
                emit_body()

    nc.compile()
    return nc


def _route(xf: np.ndarray, router_w: np.ndarray):
    """Top-2 routing + softmax weights, fp32, matching the jax reference."""
    T = xf.shape[0]
    logits = xf @ router_w  # [T, E]
    rows = np.arange(T)
    i1 = logits.argmax(axis=1)
    tmp = logits.copy()
    tmp[rows, i1] = -np.inf
    i2 = tmp.argmax(axis=1)
    v1 = logits[rows, i1]
    v2 = tmp[rows, i2]
    e2 = np.exp((v2 - v1).astype(np.float32))
    g1 = 1.0 / (1.0 + e2)
    g2 = e2 / (1.0 + e2)
    return i1, i2, g1.astype(np.float32), g2.astype(np.float32)


def _prepare(x, router_w, w1, w2, w3):
    """Route + dispatch on host; returns (C, in_maps, idxs, shape)."""
    x = np.asarray(x, dtype=np.float32)
    router_w = np.asarray(router_w, dtype=np.float32)
    w1 = np.asarray(w1, dtype=np.float32)
    w2 = np.asarray(w2, dtype=np.float32)
    w3 = np.asarray(w3, dtype=np.float32)

    B, S, D = x.shape
    T = B * S
    xf = x.reshape(T, D)

    i1, i2, g1, g2 = _route(xf, router_w)

    # per-expert token lists (slot-0 tokens then slot-1 tokens)
    idxs, wgts = [], []
    for e in range(NUM_EXPERTS):
        s0 = np.nonzero(i1 == e)[0]
        s1 = np.nonzero(i2 == e)[0]
        idxs.append(np.concatenate([s0, s1]))
        wgts.append(np.concatenate([g1[s0], g2[s1]]))
    max_cnt = max(len(ix) for ix in idxs)
    C = max(P, ((max_cnt + P - 1) // P) * P)

    bf16 = ml_dtypes.bfloat16
    # expert weights, padded along the hidden dim and cast to bf16
    w1p = np.zeros((NUM_EXPERTS, D, HP), dtype=bf16)
    w1p[:, :, :HIDDEN] = w1
    w2p = np.zeros((NUM_EXPERTS, D, HP), dtype=bf16)
    w2p[:, :, :HIDDEN] = w2
    w3p = np.zeros((NUM_EXPERTS, HP, D), dtype=bf16)
    w3p[:, :HIDDEN, :] = w3

    in_maps = []
    for e in range(NUM_EXPERTS):
        ix = idxs[e]
        xg = np.zeros((C, D), dtype=np.float32)
        xg[: len(ix)] = xf[ix]
        wvec = np.zeros((C,), dtype=np.float32)
        wvec[: len(ix)] = wgts[e]
        wvec = np.ascontiguousarray(wvec.reshape(C // P, P).T)  # [P, C//P]
        in_maps.append(
            {
                "xt": np.ascontiguousarray(xg.T).astype(bf16).reshape(KD, P, C),
                "w1": w1p[e].reshape(KD, P, HP),
                "w2": w2p[e].reshape(KD, P, HP),
                "w3": w3p[e].reshape(HPT, P, DIM),
                "wv": wvec,
            }
        )
    return C, in_maps, idxs, (B, S, D)


def kernel(x, router_w, w1, w2, w3):
    global LAST_RESULT
    from concourse.bass_utils import run_bass_kernel_spmd

    C, in_maps, idxs, (B, S, D) = _prepare(x, router_w, w1, w2, w3)

    if C not in _KERNELS:
        _KERNELS[C] = _build(C)
    nc = _KERNELS[C]

    res = run_bass_kernel_spmd(
        nc,
        in_maps,
        list(range(NUM_EXPERTS)),
        trace=TRACE,
    )
    LAST_RESULT = res

    out = np.zeros((B * S, D), dtype=np.float32)
    for e in range(NUM_EXPERTS):
        ix = idxs[e]
        out[ix] += res.results[e]["y"][: len(ix)]
    return out.reshape(B, S, D)


# revision 27
# speedup vs baseline: 1.0416x; 1.0416x over previous
"""MoE layer (8 experts, top-2, SwiGLU) for Trainium2, expert-parallel over 8 cores.

Strategy:
  - Router (x @ router_w, top-2, softmax) runs on host in fp32 — it is 0.01%
    of the FLOPs and determines the (data-dependent) sharding.
  - Each core is assigned one expert. Tokens routed to that expert are
    gathered on host, padded to a common capacity C, and shipped transposed
    as xT [D, C] so both GEMMs need no on-device transpose:
        h1T = w1.T @ x.T   (lhsT = w1 [D,Hp], rhs = xT [D,C])   -> [Hp, C]
        h2T = w2.T @ x.T
        hT  = silu(h1T) * h2T
        y   = hT.T @ w3    (lhsT = hT [Hp,C], rhs = w3 [Hp,D])  -> [C, D]
    y rows are scaled by the per-token combine weight on device.
  - Host scatter-adds the 8 per-expert outputs back to [B,S,D].

  Matmuls run in bf16 (fp32 accumulate in PSUM); hidden dim 2730 is padded
  to 2816 = 22*128 (zero pad is exact: silu(0)*0 = 0).

  mm3 computes its two 512-wide output halves in parallel PSUM banks so
  each h-tile LDWEIGHTS is shared between them. (An analogous pairing of
  token blocks for mm1/mm2 A/B-benched neutral on HW and is off by default.)
"""

import os

import numpy as np
import ml_dtypes

DIM = 1024
NUM_EXPERTS = 8
HIDDEN = 2730
P = 128
HP = 2816  # hidden padded to 22*128
KD = DIM // P  # 8 contraction chunks for mm1/mm2
HPT = HP // P  # 22 chunks of the hidden dim
NBLK = 512  # token block (moving free dim per matmul)

TRACE = os.environ.get("MOE_TRACE", "0") == "1"
LAST_RESULT = None  # BassKernelResults of the last run (for test harness)

_KERNELS: dict = {}


def _build(C: int, reps: int = 1, pair: bool = False):
    """Build + compile the per-core Bass kernel for capacity C (multiple of 128).

    reps > 1 wraps the whole body (including DMAs) in a device-side loop that
    recomputes the same result `reps` times — only used for wall-clock
    benchmarking (dispatch overhead cancels in the rep delta).
    """
    import concourse.mybir as mybir
    import concourse.tile as tile
    from concourse import bacc

    dt = mybir.dt
    nc = bacc.Bacc(None, target_bir_lowering=False)

    xt = nc.dram_tensor("xt", [KD, P, C], dt.bfloat16, kind="ExternalInput")
    w1 = nc.dram_tensor("w1", [KD, P, HP], dt.bfloat16, kind="ExternalInput")
    w2 = nc.dram_tensor("w2", [KD, P, HP], dt.bfloat16, kind="ExternalInput")
    w3 = nc.dram_tensor("w3", [HPT, P, DIM], dt.bfloat16, kind="ExternalInput")
    wv = nc.dram_tensor("wv", [P, C // P], dt.float32, kind="ExternalInput")
    y = nc.dram_tensor("y", [C, DIM], dt.float32, kind="ExternalOutput")

    blocks = []
    c0 = 0
    while c0 < C:
        bn = min(NBLK, C - c0)
        blocks.append((c0, bn))
        c0 += bn
    # group full-size blocks into pairs (mm1/mm2 share each LDWEIGHTS
    # between the two blocks of a pair)
    pairs = []
    i = 0
    while i < len(blocks):
        if (
            pair
            and i + 1 < len(blocks)
            and blocks[i][1] == NBLK
            and blocks[i + 1][1] == NBLK
        ):
            pairs.append((blocks[i], blocks[i + 1]))
            i += 2
        else:
            pairs.append((blocks[i],))
            i += 1
    nbuf = 1 if pair else 2

    with tile.TileContext(nc) as tc:
        with (
            tc.tile_pool(name="wpool", bufs=1) as wpool,
            tc.tile_pool(name="xpool", bufs=nbuf) as xpool,
            tc.tile_pool(name="hpool", bufs=1) as hpool,
            tc.tile_pool(name="tpool", bufs=2) as tpool,
            tc.tile_pool(name="ypool", bufs=3) as ypool,
            tc.tile_pool(name="psA", bufs=nbuf, space="PSUM") as psA,
            tc.tile_pool(name="psB", bufs=nbuf, space="PSUM") as psB,
            tc.tile_pool(name="psC", bufs=2, space="PSUM") as psC,
        ):

            def make_x(pi, bj, c0, bn):
                t = xpool.tile(
                    [P, KD, NBLK], dt.bfloat16, name=f"x_{pi}_{bj}", tag=f"x_{bj}"
                )
                for kd in range(KD):
                    nc.sync.dma_start(t[:, kd, :bn], xt[kd][:, c0 : c0 + bn])
                return t

            def emit_body():
                # First pair's activations, so mm1 can start early.
                x0 = [
                    make_x(0, bj, c0, bn) for bj, (c0, bn) in enumerate(pairs[0])
                ]

                # Resident weights, DMA'd in hp-sliced parts in the order the
                # first pair's matmuls consume them, split across the SP HWDGE
                # queue (x, w1, w3) and the gpsimd SWDGE queue (w2, wv).
                w1_sb = [
                    wpool.tile([P, HP], dt.bfloat16, name=f"w1_{kd}", tag=f"w1_{kd}")
                    for kd in range(KD)
                ]
                w2_sb = [
                    wpool.tile([P, HP], dt.bfloat16, name=f"w2_{kd}", tag=f"w2_{kd}")
                    for kd in range(KD)
                ]
                w3_sb = [
                    wpool.tile([P, DIM], dt.bfloat16, name=f"w3_{hp}", tag=f"w3_{hp}")
                    for hp in range(HPT)
                ]
                bounds = [0, 3 * P, 7 * P, 12 * P, 17 * P, HP]
                for pi in range(len(bounds) - 1):
                    sl = slice(bounds[pi], bounds[pi + 1])
                    for kd in range(KD):
                        nc.sync.dma_start(w1_sb[kd][:, sl], w1[kd][:, sl])
                    for kd in range(KD):
                        nc.gpsimd.dma_start(w2_sb[kd][:, sl], w2[kd][:, sl])
                for hp in range(HPT):
                    nc.sync.dma_start(w3_sb[hp][:], w3[hp])

                wv_sb = wpool.tile([P, C // P], dt.float32, name="wv_sb", tag="wv_sb")
                nc.gpsimd.dma_start(wv_sb[:], wv[:])

                def mm3_block(pi, bj, c0, bn, h_sb):
                    # y[block] = (hT.T @ w3) * combine_weight; the two 512-wide
                    # output halves accumulate in parallel banks sharing LDW.
                    for cs in range(bn // P):
                        ci = c0 // P + cs
                        ps3 = [
                            psC.tile(
                                [P, 512],
                                dt.float32,
                                name=f"ps3_{pi}_{bj}_{cs}_{dti}",
                                tag=f"ps3_{dti}",
                            )
                            for dti in range(DIM // 512)
                        ]
                        for hp in range(HPT):
                            lhsT = h_sb[hp][:, cs * P : (cs + 1) * P]
                            for dti in range(DIM // 512):
                                nc.tensor.matmul(
                                    ps3[dti][:],
                                    lhsT,
                                    w3_sb[hp][:, dti * 512 : (dti + 1) * 512],
                                    start=(hp == 0),
                                    stop=(hp == HPT - 1),
                                )
                        for dti in range(DIM // 512):
                            yt = ypool.tile(
                                [P, 512],
                                dt.float32,
                                name=f"y_{pi}_{bj}_{cs}_{dti}",
                                tag="yt",
                            )
                            nc.vector.tensor_scalar_mul(
                                yt[:], ps3[dti][:], wv_sb[:, ci : ci + 1]
                            )
                            nc.sync.dma_start(
                                y[
                                    c0 + cs * P : c0 + (cs + 1) * P,
                                    dti * 512 : (dti + 1) * 512,
                                ],
                                yt[:],
                            )

                for pi, pblocks in enumerate(pairs):
                    nb = len(pblocks)
                    if pi == 0:
                        xb = x0
                    else:
                        xb = [
                            make_x(pi, bj, c0, bn)
                            for bj, (c0, bn) in enumerate(pblocks)
                        ]

                    # hT = silu(w1.T @ xT) * (w2.T @ xT) for each block of the
                    # pair; both blocks' matmuls share each LDWEIGHTS.
                    h_sbs = [[] for _ in range(nb)]
                    for hp in range(HPT):
                        ps1 = [
                            psA.tile(
                                [P, pblocks[bj][1]],
                                dt.float32,
                                name=f"ps1_{pi}_{bj}_{hp}",
                                tag=f"ps1_{bj}",
                            )
                            for bj in range(nb)
                        ]
                        for kd in range(KD):
                            lhsT = w1_sb[kd][:, hp * P : (hp + 1) * P]
                            for bj in range(nb):
                                nc.tensor.matmul(
                                    ps1[bj][:],
                                    lhsT,
                                    xb[bj][:, kd, 0 : pblocks[bj][1]],
                                    start=(kd == 0),
                                    stop=(kd == KD - 1),
                                )
                        tsil = []
                        for bj in range(nb):
                            t = tpool.tile(
                                [P, pblocks[bj][1]],
                                dt.bfloat16,
                                name=f"sil_{pi}_{bj}_{hp}",
                                tag=f"sil_{bj}",
                            )
                            nc.scalar.activation(
                                t[:], ps1[bj][:], mybir.ActivationFunctionType.Silu
                            )
                            tsil.append(t)
                        ps2 = [
                            psB.tile(
                                [P, pblocks[bj][1]],
                                dt.float32,
                                name=f"ps2_{pi}_{bj}_{hp}",
                                tag=f"ps2_{bj}",
                            )
                            for bj in range(nb)
                        ]
                        for kd in range(KD):
                            lhsT = w2_sb[kd][:, hp * P : (hp + 1) * P]
                            for bj in range(nb):
                                nc.tensor.matmul(
                                    ps2[bj][:],
                                    lhsT,
                                    xb[bj][:, kd, 0 : pblocks[bj][1]],
                                    start=(kd == 0),
                                    stop=(kd == KD - 1),
                                )
                        for bj in range(nb):
                            ht = hpool.tile(
                                [P, pblocks[bj][1]],
                                dt.bfloat16,
                                name=f"h_{pi}_{bj}_{hp}",
                                tag=f"h_{hp}_{bj}",
                            )
                            nc.vector.tensor_mul(ht[:], tsil[bj][:], ps2[bj][:])
                            h_sbs[bj].append(ht)

                    for bj, (c0, bn) in enumerate(pblocks):
                        mm3_block(pi, bj, c0, bn, h_sbs[bj])

            if reps > 1:
                with tc.For_i(0, reps, 1):
                    emit_body()
            else:
                emit_body()

    nc.compile()
    return nc


def _route(xf: np.ndarray, router_w: np.ndarray):
    """Top-2 routing + softmax weights, fp32, matching the jax reference."""
    T = xf.shape[0]
    logits = xf @ router_w  # [T, E]
    rows = np.arange(T)
    i1 = logits.argmax(axis=1)
    tmp = logits.copy()
    tmp[rows, i1] = -np.inf
    i2 = tmp.argmax(axis=1)
    v1 = logits[rows, i1]
    v2 = tmp[rows, i2]
    e2 = np.exp((v2 - v1).astype(np.float32))
    g1 = 1.0 / (1.0 + e2)
    g2 = e2 / (1.0 + e2)
    return i1, i2, g1.astype(np.float32), g2.astype(np.float32)


def _prepare(x, router_w, w1, w2, w3):
    """Route + dispatch on host; returns (C, in_maps, idxs, shape)."""
    x = np.asarray(x, dtype=np.float32)
    router_w = np.asarray(router_w, dtype=np.float32)
    w1 = np.asarray(w1, dtype=np.float32)
    w2 = np.asarray(w2, dtype=np.float32)
    w3 = np.asarray(w3, dtype=np.float32)

    B, S, D = x.shape
    T = B * S
    xf = x.reshape(T, D)

    i1, i2, g1, g2 = _route(xf, router_w)

    # per-expert token lists (slot-0 tokens then slot-1 tokens)
    idxs, wgts = [], []
    for e in range(NUM_EXPERTS):
        s0 = np.nonzero(i1 == e)[0]
        s1 = np.nonzero(i2 == e)[0]
        idxs.append(np.concatenate([s0, s1]))
        wgts.append(np.concatenate([g1[s0], g2[s1]]))
    max_cnt = max(len(ix) for ix in idxs)
    C = max(P, ((max_cnt + P - 1) // P) * P)

    bf16 = ml_dtypes.bfloat16
    # expert weights, padded along the hidden dim and cast to bf16
    w1p = np.zeros((NUM_EXPERTS, D, HP), dtype=bf16)
    w1p[:, :, :HIDDEN] = w1
    w2p = np.zeros((NUM_EXPERTS, D, HP), dtype=bf16)
    w2p[:, :, :HIDDEN] = w2
    w3p = np.zeros((NUM_EXPERTS, HP, D), dtype=bf16)
    w3p[:, :HIDDEN, :] = w3

    in_maps = []
    for e in range(NUM_EXPERTS):
        ix = idxs[e]
        xg = np.zeros((C, D), dtype=np.float32)
        xg[: len(ix)] = xf[ix]
        wvec = np.zeros((C,), dtype=np.float32)
        wvec[: len(ix)] = wgts[e]
        wvec = np.ascontiguousarray(wvec.reshape(C // P, P).T)  # [P, C//P]
        in_maps.append(
            {
                "xt": np.ascontiguousarray(xg.T).astype(bf16).reshape(KD, P, C),
                "w1": w1p[e].reshape(KD, P, HP),
                "w2": w2p[e].reshape(KD, P, HP),
                "w3": w3p[e].reshape(HPT, P, DIM),
                "wv": wvec,
            }
        )
    return C, in_maps, idxs, (B, S, D)


def kernel(x, router_w, w1, w2, w3):
    global LAST_RESULT
    from concourse.bass_utils import run_bass_kernel_spmd

    C, in_maps, idxs, (B, S, D) = _prepare(x, router_w, w1, w2, w3)

    if C not in _KERNELS:
        _KERNELS[C] = _build(C)
    nc = _KERNELS[C]

    res = run_bass_kernel_spmd(
        nc,
        in_maps,
        list(range(NUM_EXPERTS)),
        trace=TRACE,
    )
    LAST_RESULT = res

    out = np.zeros((B * S, D), dtype=np.float32)
    for e in range(NUM_EXPERTS):
        ix = idxs[e]
        out[ix] += res.results[e]["y"][: len(ix)]
    return out.reshape(B, S, D)


# revision 29
# speedup vs baseline: 1.0490x; 1.0072x over previous
"""MoE layer (8 experts, top-2, SwiGLU) for Trainium2, expert-parallel over 8 cores.

Strategy:
  - Router (x @ router_w, top-2, softmax) runs on host in fp32 — it is 0.01%
    of the FLOPs and determines the (data-dependent) sharding.
  - Each core is assigned one expert. Tokens routed to that expert are
    gathered on host, padded to a common capacity C, and shipped transposed
    as xT [D, C] so both GEMMs need no on-device transpose:
        h1T = w1.T @ x.T   (lhsT = w1 [D,Hp], rhs = xT [D,C])   -> [Hp, C]
        h2T = w2.T @ x.T
        hT  = silu(h1T) * h2T
        y   = hT.T @ w3    (lhsT = hT [Hp,C], rhs = w3 [Hp,D])  -> [C, D]
    y rows are scaled by the per-token combine weight on device.
  - Host scatter-adds the 8 per-expert outputs back to [B,S,D].

  Matmuls run in bf16 (fp32 accumulate in PSUM); hidden dim 2730 is padded
  to 2816 = 22*128 (zero pad is exact: silu(0)*0 = 0).

  mm3 computes its two 512-wide output halves in parallel PSUM banks so
  each h-tile LDWEIGHTS is shared between them. (An analogous pairing of
  token blocks for mm1/mm2 A/B-benched neutral on HW and is off by default.)
"""

import os

import numpy as np
import ml_dtypes

DIM = 1024
NUM_EXPERTS = 8
HIDDEN = 2730
P = 128
HP = 2816  # hidden padded to 22*128
KD = DIM // P  # 8 contraction chunks for mm1/mm2
HPT = HP // P  # 22 chunks of the hidden dim
NBLK = 512  # token block (moving free dim per matmul)

TRACE = os.environ.get("MOE_TRACE", "0") == "1"
LAST_RESULT = None  # BassKernelResults of the last run (for test harness)

_KERNELS: dict = {}


def _build(C: int, c_real: int | None = None, reps: int = 1, pair: bool = False):
    """Build + compile the per-core Bass kernel for capacity C (multiple of 128).

    reps > 1 wraps the whole body (including DMAs) in a device-side loop that
    recomputes the same result `reps` times — only used for wall-clock
    benchmarking (dispatch overhead cancels in the rep delta).
    """
    import concourse.mybir as mybir
    import concourse.tile as tile
    from concourse import bacc

    dt = mybir.dt
    nc = bacc.Bacc(None, target_bir_lowering=False)

    xt = nc.dram_tensor("xt", [KD, P, C], dt.bfloat16, kind="ExternalInput")
    w1 = nc.dram_tensor("w1", [KD, P, HP], dt.bfloat16, kind="ExternalInput")
    w2 = nc.dram_tensor("w2", [KD, P, HP], dt.bfloat16, kind="ExternalInput")
    w3 = nc.dram_tensor("w3", [HPT, P, DIM], dt.bfloat16, kind="ExternalInput")
    wv = nc.dram_tensor("wv", [P, C // P], dt.float32, kind="ExternalInput")
    y = nc.dram_tensor("y", [C, DIM], dt.float32, kind="ExternalOutput")

    # Only c_real tokens are real; rows beyond that are padding whose
    # output the host ignores, so the last block shrinks to the real count.
    if c_real is None:
        c_real = C
    blocks = []
    c0 = 0
    while c0 < c_real:
        bn = min(NBLK, c_real - c0)
        blocks.append((c0, bn))
        c0 += bn
    # group full-size blocks into pairs (mm1/mm2 share each LDWEIGHTS
    # between the two blocks of a pair)
    pairs = []
    i = 0
    while i < len(blocks):
        if (
            pair
            and i + 1 < len(blocks)
            and blocks[i][1] == NBLK
            and blocks[i + 1][1] == NBLK
        ):
            pairs.append((blocks[i], blocks[i + 1]))
            i += 2
        else:
            pairs.append((blocks[i],))
            i += 1
    nbuf = 1 if pair else 2

    with tile.TileContext(nc) as tc:
        with (
            tc.tile_pool(name="wpool", bufs=1) as wpool,
            tc.tile_pool(name="xpool", bufs=nbuf) as xpool,
            tc.tile_pool(name="hpool", bufs=1) as hpool,
            tc.tile_pool(name="tpool", bufs=2) as tpool,
            tc.tile_pool(name="ypool", bufs=3) as ypool,
            tc.tile_pool(name="psA", bufs=nbuf, space="PSUM") as psA,
            tc.tile_pool(name="psB", bufs=nbuf, space="PSUM") as psB,
            tc.tile_pool(name="psC", bufs=2, space="PSUM") as psC,
        ):

            def make_x(pi, bj, c0, bn):
                t = xpool.tile(
                    [P, KD, NBLK], dt.bfloat16, name=f"x_{pi}_{bj}", tag=f"x_{bj}"
                )
                for kd in range(KD):
                    nc.sync.dma_start(t[:, kd, :bn], xt[kd][:, c0 : c0 + bn])
                return t

            def emit_body():
                # First pair's activations, so mm1 can start early.
                x0 = [
                    make_x(0, bj, c0, bn) for bj, (c0, bn) in enumerate(pairs[0])
                ]

                # Resident weights, DMA'd in hp-sliced parts in the order the
                # first pair's matmuls consume them, split across the SP HWDGE
                # queue (x, w1, w3) and the gpsimd SWDGE queue (w2, wv).
                w1_sb = [
                    wpool.tile([P, HP], dt.bfloat16, name=f"w1_{kd}", tag=f"w1_{kd}")
                    for kd in range(KD)
                ]
                w2_sb = [
                    wpool.tile([P, HP], dt.bfloat16, name=f"w2_{kd}", tag=f"w2_{kd}")
                    for kd in range(KD)
                ]
                w3_sb = [
                    wpool.tile([P, DIM], dt.bfloat16, name=f"w3_{hp}", tag=f"w3_{hp}")
                    for hp in range(HPT)
                ]
                bounds = [0, 3 * P, 7 * P, 12 * P, 17 * P, HP]
                for pi in range(len(bounds) - 1):
                    sl = slice(bounds[pi], bounds[pi + 1])
                    for kd in range(KD):
                        nc.sync.dma_start(w1_sb[kd][:, sl], w1[kd][:, sl])
                    for kd in range(KD):
                        nc.gpsimd.dma_start(w2_sb[kd][:, sl], w2[kd][:, sl])
                for hp in range(HPT):
                    nc.sync.dma_start(w3_sb[hp][:], w3[hp])

                wv_sb = wpool.tile([P, C // P], dt.float32, name="wv_sb", tag="wv_sb")
                nc.gpsimd.dma_start(wv_sb[:], wv[:])

                def mm3_block(pi, bj, c0, bn, h_sb):
                    # y[block] = (hT.T @ w3) * combine_weight; the two 512-wide
                    # output halves accumulate in parallel banks sharing LDW.
                    for cs in range((bn + P - 1) // P):
                        M = min(P, bn - cs * P)
                        ci = c0 // P + cs
                        ps3 = [
                            psC.tile(
                                [P, 512],
                                dt.float32,
                                name=f"ps3_{pi}_{bj}_{cs}_{dti}",
                                tag=f"ps3_{dti}",
                            )
                            for dti in range(DIM // 512)
                        ]
                        for hp in range(HPT):
                            lhsT = h_sb[hp][:, cs * P : cs * P + M]
                            for dti in range(DIM // 512):
                                nc.tensor.matmul(
                                    ps3[dti][:M],
                                    lhsT,
                                    w3_sb[hp][:, dti * 512 : (dti + 1) * 512],
                                    start=(hp == 0),
                                    stop=(hp == HPT - 1),
                                )
                        for dti in range(DIM // 512):
                            yt = ypool.tile(
                                [P, 512],
                                dt.float32,
                                name=f"y_{pi}_{bj}_{cs}_{dti}",
                                tag="yt",
                            )
                            nc.vector.tensor_scalar_mul(
                                yt[:M], ps3[dti][:M], wv_sb[:M, ci : ci + 1]
                            )
                            nc.sync.dma_start(
                                y[
                                    c0 + cs * P : c0 + cs * P + M,
                                    dti * 512 : (dti + 1) * 512,
                                ],
                                yt[:M],
                            )

                for pi, pblocks in enumerate(pairs):
                    nb = len(pblocks)
                    if pi == 0:
                        xb = x0
                    else:
                        xb = [
                            make_x(pi, bj, c0, bn)
                            for bj, (c0, bn) in enumerate(pblocks)
                        ]

                    # hT = silu(w1.T @ xT) * (w2.T @ xT) for each block of the
                    # pair; both blocks' matmuls share each LDWEIGHTS.
                    h_sbs = [[] for _ in range(nb)]
                    for hp in range(HPT):
                        ps1 = [
                            psA.tile(
                                [P, pblocks[bj][1]],
                                dt.float32,
                                name=f"ps1_{pi}_{bj}_{hp}",
                                tag=f"ps1_{bj}",
                            )
                            for bj in range(nb)
                        ]
                        for kd in range(KD):
                            lhsT = w1_sb[kd][:, hp * P : (hp + 1) * P]
                            for bj in range(nb):
                                nc.tensor.matmul(
                                    ps1[bj][:],
                                    lhsT,
                                    xb[bj][:, kd, 0 : pblocks[bj][1]],
                                    start=(kd == 0),
                                    stop=(kd == KD - 1),
                                )
                        tsil = []
                        for bj in range(nb):
                            t = tpool.tile(
                                [P, pblocks[bj][1]],
                                dt.float32,
                                name=f"sil_{pi}_{bj}_{hp}",
                                tag=f"sil_{bj}",
                            )
                            nc.scalar.activation(
                                t[:], ps1[bj][:], mybir.ActivationFunctionType.Silu
                            )
                            tsil.append(t)
                        ps2 = [
                            psB.tile(
                                [P, pblocks[bj][1]],
                                dt.float32,
                                name=f"ps2_{pi}_{bj}_{hp}",
                                tag=f"ps2_{bj}",
                            )
                            for bj in range(nb)
                        ]
                        for kd in range(KD):
                            lhsT = w2_sb[kd][:, hp * P : (hp + 1) * P]
                            for bj in range(nb):
                                nc.tensor.matmul(
                                    ps2[bj][:],
                                    lhsT,
                                    xb[bj][:, kd, 0 : pblocks[bj][1]],
                                    start=(kd == 0),
                                    stop=(kd == KD - 1),
                                )
                        for bj in range(nb):
                            ht = hpool.tile(
                                [P, pblocks[bj][1]],
                                dt.bfloat16,
                                name=f"h_{pi}_{bj}_{hp}",
                                tag=f"h_{hp}_{bj}",
                            )
                            nc.vector.tensor_mul(ht[:], tsil[bj][:], ps2[bj][:])
                            h_sbs[bj].append(ht)

                    for bj, (c0, bn) in enumerate(pblocks):
                        mm3_block(pi, bj, c0, bn, h_sbs[bj])

            if reps > 1:
                with tc.For_i(0, reps, 1):
                    emit_body()
            else:
                emit_body()

    nc.compile()
    return nc


def _route(xf: np.ndarray, router_w: np.ndarray):
    """Top-2 routing + softmax weights, fp32, matching the jax reference."""
    T = xf.shape[0]
    logits = xf @ router_w  # [T, E]
    rows = np.arange(T)
    i1 = logits.argmax(axis=1)
    tmp = logits.copy()
    tmp[rows, i1] = -np.inf
    i2 = tmp.argmax(axis=1)
    v1 = logits[rows, i1]
    v2 = tmp[rows, i2]
    e2 = np.exp((v2 - v1).astype(np.float32))
    g1 = 1.0 / (1.0 + e2)
    g2 = e2 / (1.0 + e2)
    return i1, i2, g1.astype(np.float32), g2.astype(np.float32)


def _prepare(x, router_w, w1, w2, w3):
    """Route + dispatch on host; returns (C, in_maps, idxs, shape)."""
    x = np.asarray(x, dtype=np.float32)
    router_w = np.asarray(router_w, dtype=np.float32)
    w1 = np.asarray(w1, dtype=np.float32)
    w2 = np.asarray(w2, dtype=np.float32)
    w3 = np.asarray(w3, dtype=np.float32)

    B, S, D = x.shape
    T = B * S
    xf = x.reshape(T, D)

    i1, i2, g1, g2 = _route(xf, router_w)

    # per-expert token lists (slot-0 tokens then slot-1 tokens)
    idxs, wgts = [], []
    for e in range(NUM_EXPERTS):
        s0 = np.nonzero(i1 == e)[0]
        s1 = np.nonzero(i2 == e)[0]
        idxs.append(np.concatenate([s0, s1]))
        wgts.append(np.concatenate([g1[s0], g2[s1]]))
    max_cnt = max(len(ix) for ix in idxs)
    C = max(P, ((max_cnt + P - 1) // P) * P)

    bf16 = ml_dtypes.bfloat16
    # expert weights, padded along the hidden dim and cast to bf16
    w1p = np.zeros((NUM_EXPERTS, D, HP), dtype=bf16)
    w1p[:, :, :HIDDEN] = w1
    w2p = np.zeros((NUM_EXPERTS, D, HP), dtype=bf16)
    w2p[:, :, :HIDDEN] = w2
    w3p = np.zeros((NUM_EXPERTS, HP, D), dtype=bf16)
    w3p[:, :HIDDEN, :] = w3

    in_maps = []
    for e in range(NUM_EXPERTS):
        ix = idxs[e]
        xg = np.zeros((C, D), dtype=np.float32)
        xg[: len(ix)] = xf[ix]
        wvec = np.zeros((C,), dtype=np.float32)
        wvec[: len(ix)] = wgts[e]
        wvec = np.ascontiguousarray(wvec.reshape(C // P, P).T)  # [P, C//P]
        in_maps.append(
            {
                "xt": np.ascontiguousarray(xg.T).astype(bf16).reshape(KD, P, C),
                "w1": w1p[e].reshape(KD, P, HP),
                "w2": w2p[e].reshape(KD, P, HP),
                "w3": w3p[e].reshape(HPT, P, DIM),
                "wv": wvec,
            }
        )
    return C, in_maps, idxs, (B, S, D)


def kernel(x, router_w, w1, w2, w3):
    global LAST_RESULT
    from concourse.bass_utils import run_bass_kernel_spmd

    C, in_maps, idxs, (B, S, D) = _prepare(x, router_w, w1, w2, w3)

    max_cnt = max(len(ix) for ix in idxs)
    key = (C, max_cnt)
    if key not in _KERNELS:
        _KERNELS[key] = _build(C, c_real=max_cnt)
    nc = _KERNELS[key]

    res = run_bass_kernel_spmd(
        nc,
        in_maps,
        list(range(NUM_EXPERTS)),
        trace=TRACE,
    )
    LAST_RESULT = res

    out = np.zeros((B * S, D), dtype=np.float32)
    for e in range(NUM_EXPERTS):
        ix = idxs[e]
        out[ix] += res.results[e]["y"][: len(ix)]
    return out.reshape(B, S, D)


# revision 30
# speedup vs baseline: 2.2177x; 2.1140x over previous
"""MoE layer (8 experts, top-2, SwiGLU) for Trainium2, expert-parallel over 8 cores.

Strategy:
  - Router (x @ router_w, top-2, softmax) runs on host in fp32 — it is 0.01%
    of the FLOPs and determines the (data-dependent) sharding.
  - Each core is assigned one expert. Tokens routed to that expert are
    gathered on host, padded to a common capacity C, and shipped transposed
    as xT [D, C] so both GEMMs need no on-device transpose:
        h1T = w1.T @ x.T   (lhsT = w1 [D,Hp], rhs = xT [D,C])   -> [Hp, C]
        h2T = w2.T @ x.T
        hT  = silu(h1T) * h2T
        y   = hT.T @ w3    (lhsT = hT [Hp,C], rhs = w3 [Hp,D])  -> [C, D]
    y rows are scaled by the per-token combine weight on device.
  - Host scatter-adds the 8 per-expert outputs back to [B,S,D].

  Matmuls run in bf16 (fp32 accumulate in PSUM); hidden dim 2730 is padded
  to 2816 = 22*128 (zero pad is exact: silu(0)*0 = 0).

  mm3 computes its two 512-wide output halves in parallel PSUM banks so
  each h-tile LDWEIGHTS is shared between them. (An analogous pairing of
  token blocks for mm1/mm2 A/B-benched neutral on HW and is off by default.)
"""

import os

import numpy as np
import ml_dtypes

DIM = 1024
NUM_EXPERTS = 8
HIDDEN = 2730
P = 128
HP = 2816  # hidden padded to 22*128
KD = DIM // P  # 8 contraction chunks for mm1/mm2
HPT = HP // P  # 22 chunks of the hidden dim
NBLK = 512  # token block (moving free dim per matmul)

TRACE = os.environ.get("MOE_TRACE", "0") == "1"
LAST_RESULT = None  # BassKernelResults of the last run (for test harness)

_KERNELS: dict = {}


def _build(C: int, c_real: int | None = None, reps: int = 1, pair: bool = False):
    """Build + compile the per-core Bass kernel for capacity C (multiple of 128).

    reps > 1 wraps the whole body (including DMAs) in a device-side loop that
    recomputes the same result `reps` times — only used for wall-clock
    benchmarking (dispatch overhead cancels in the rep delta).
    """
    import concourse.mybir as mybir
    import concourse.tile as tile
    from concourse import bacc

    dt = mybir.dt
    nc = bacc.Bacc(None, target_bir_lowering=False)

    xt = nc.dram_tensor("xt", [KD, P, C], dt.bfloat16, kind="ExternalInput")
    w1 = nc.dram_tensor("w1", [KD, P, HP], dt.bfloat16, kind="ExternalInput")
    w2 = nc.dram_tensor("w2", [KD, P, HP], dt.bfloat16, kind="ExternalInput")
    w3 = nc.dram_tensor("w3", [HPT, P, DIM], dt.bfloat16, kind="ExternalInput")
    wv = nc.dram_tensor("wv", [P, C // P], dt.float32, kind="ExternalInput")
    y = nc.dram_tensor("y", [C, DIM], dt.float32, kind="ExternalOutput")

    # Only c_real tokens are real; rows beyond that are padding whose
    # output the host ignores, so the last block shrinks to the real count.
    if c_real is None:
        c_real = C
    blocks = []
    c0 = 0
    while c0 < c_real:
        bn = min(NBLK, c_real - c0)
        blocks.append((c0, bn))
        c0 += bn
    # group full-size blocks into pairs (mm1/mm2 share each LDWEIGHTS
    # between the two blocks of a pair)
    pairs = []
    i = 0
    while i < len(blocks):
        if (
            pair
            and i + 1 < len(blocks)
            and blocks[i][1] == NBLK
            and blocks[i + 1][1] == NBLK
        ):
            pairs.append((blocks[i], blocks[i + 1]))
            i += 2
        else:
            pairs.append((blocks[i],))
            i += 1
    nbuf = 1 if pair else 2

    with tile.TileContext(nc) as tc:
        with (
            tc.tile_pool(name="wpool", bufs=1) as wpool,
            tc.tile_pool(name="xpool", bufs=nbuf) as xpool,
            tc.tile_pool(name="hpool", bufs=1) as hpool,
            tc.tile_pool(name="tpool", bufs=2) as tpool,
            tc.tile_pool(name="ypool", bufs=3) as ypool,
            tc.tile_pool(name="psA", bufs=nbuf + 1, space="PSUM") as psA,
            tc.tile_pool(name="psB", bufs=nbuf + 1, space="PSUM") as psB,
            tc.tile_pool(name="psC", bufs=1, space="PSUM") as psC,
        ):

            def make_x(pi, bj, c0, bn):
                t = xpool.tile(
                    [P, KD, NBLK], dt.bfloat16, name=f"x_{pi}_{bj}", tag=f"x_{bj}"
                )
                for kd in range(KD):
                    nc.sync.dma_start(t[:, kd, :bn], xt[kd][:, c0 : c0 + bn])
                return t

            def emit_body():
                # First pair's activations, so mm1 can start early.
                x0 = [
                    make_x(0, bj, c0, bn) for bj, (c0, bn) in enumerate(pairs[0])
                ]

                # Resident weights, DMA'd in hp-sliced parts in the order the
                # first pair's matmuls consume them, split across the SP HWDGE
                # queue (x, w1, w3) and the gpsimd SWDGE queue (w2, wv).
                w1_sb = [
                    wpool.tile([P, HP], dt.bfloat16, name=f"w1_{kd}", tag=f"w1_{kd}")
                    for kd in range(KD)
                ]
                w2_sb = [
                    wpool.tile([P, HP], dt.bfloat16, name=f"w2_{kd}", tag=f"w2_{kd}")
                    for kd in range(KD)
                ]
                w3_sb = [
                    wpool.tile([P, DIM], dt.bfloat16, name=f"w3_{hp}", tag=f"w3_{hp}")
                    for hp in range(HPT)
                ]
                bounds = [0, 3 * P, 7 * P, 12 * P, 17 * P, HP]
                for pi in range(len(bounds) - 1):
                    sl = slice(bounds[pi], bounds[pi + 1])
                    for kd in range(KD):
                        nc.sync.dma_start(w1_sb[kd][:, sl], w1[kd][:, sl])
                    for kd in range(KD):
                        nc.gpsimd.dma_start(w2_sb[kd][:, sl], w2[kd][:, sl])
                for hp in range(HPT):
                    nc.sync.dma_start(w3_sb[hp][:], w3[hp])

                wv_sb = wpool.tile([P, C // P], dt.float32, name="wv_sb", tag="wv_sb")
                nc.gpsimd.dma_start(wv_sb[:], wv[:])

                def mm3_block(pi, bj, c0, bn, h_sb):
                    # y[block] = (hT.T @ w3) * combine_weight; the two 512-wide
                    # output halves accumulate in parallel banks sharing LDW.
                    for cs in range((bn + P - 1) // P):
                        M = min(P, bn - cs * P)
                        ci = c0 // P + cs
                        ps3 = [
                            psC.tile(
                                [P, 512],
                                dt.float32,
                                name=f"ps3_{pi}_{bj}_{cs}_{dti}",
                                tag=f"ps3_{dti}",
                            )
                            for dti in range(DIM // 512)
                        ]
                        for hp in range(HPT):
                            lhsT = h_sb[hp][:, cs * P : cs * P + M]
                            for dti in range(DIM // 512):
                                nc.tensor.matmul(
                                    ps3[dti][:M],
                                    lhsT,
                                    w3_sb[hp][:, dti * 512 : (dti + 1) * 512],
                                    start=(hp == 0),
                                    stop=(hp == HPT - 1),
                                )
                        for dti in range(DIM // 512):
                            yt = ypool.tile(
                                [P, 512],
                                dt.float32,
                                name=f"y_{pi}_{bj}_{cs}_{dti}",
                                tag="yt",
                            )
                            nc.vector.tensor_scalar_mul(
                                yt[:M], ps3[dti][:M], wv_sb[:M, ci : ci + 1]
                            )
                            nc.sync.dma_start(
                                y[
                                    c0 + cs * P : c0 + cs * P + M,
                                    dti * 512 : (dti + 1) * 512,
                                ],
                                yt[:M],
                            )

                for pi, pblocks in enumerate(pairs):
                    nb = len(pblocks)
                    if pi == 0:
                        xb = x0
                    else:
                        xb = [
                            make_x(pi, bj, c0, bn)
                            for bj, (c0, bn) in enumerate(pblocks)
                        ]

                    # hT = silu(w1.T @ xT) * (w2.T @ xT) for each block of the
                    # pair; both blocks' matmuls share each LDWEIGHTS.
                    h_sbs = [[] for _ in range(nb)]
                    for hp in range(HPT):
                        ps1 = [
                            psA.tile(
                                [P, pblocks[bj][1]],
                                dt.float32,
                                name=f"ps1_{pi}_{bj}_{hp}",
                                tag=f"ps1_{bj}",
                            )
                            for bj in range(nb)
                        ]
                        for kd in range(KD):
                            lhsT = w1_sb[kd][:, hp * P : (hp + 1) * P]
                            for bj in range(nb):
                                nc.tensor.matmul(
                                    ps1[bj][:],
                                    lhsT,
                                    xb[bj][:, kd, 0 : pblocks[bj][1]],
                                    start=(kd == 0),
                                    stop=(kd == KD - 1),
                                )
                        tsil = []
                        for bj in range(nb):
                            t = tpool.tile(
                                [P, pblocks[bj][1]],
                                dt.float32,
                                name=f"sil_{pi}_{bj}_{hp}",
                                tag=f"sil_{bj}",
                            )
                            nc.scalar.activation(
                                t[:], ps1[bj][:], mybir.ActivationFunctionType.Silu
                            )
                            tsil.append(t)
                        ps2 = [
                            psB.tile(
                                [P, pblocks[bj][1]],
                                dt.float32,
                                name=f"ps2_{pi}_{bj}_{hp}",
                                tag=f"ps2_{bj}",
                            )
                            for bj in range(nb)
                        ]
                        for kd in range(KD):
                            lhsT = w2_sb[kd][:, hp * P : (hp + 1) * P]
                            for bj in range(nb):
                                nc.tensor.matmul(
                                    ps2[bj][:],
                                    lhsT,
                                    xb[bj][:, kd, 0 : pblocks[bj][1]],
                                    start=(kd == 0),
                                    stop=(kd == KD - 1),
                                )
                        for bj in range(nb):
                            ht = hpool.tile(
                                [P, pblocks[bj][1]],
                                dt.bfloat16,
                                name=f"h_{pi}_{bj}_{hp}",
                                tag=f"h_{hp}_{bj}",
                            )
                            nc.vector.tensor_mul(ht[:], tsil[bj][:], ps2[bj][:])
                            h_sbs[bj].append(ht)

                    for bj, (c0, bn) in enumerate(pblocks):
                        mm3_block(pi, bj, c0, bn, h_sbs[bj])

            if reps > 1:
                with tc.For_i(0, reps, 1):
                    emit_body()
            else:
                emit_body()

    nc.compile()
    return nc


def _route(xf: np.ndarray, router_w: np.ndarray):
    """Top-2 routing + softmax weights, fp32, matching the jax reference."""
    T = xf.shape[0]
    logits = xf @ router_w  # [T, E]
    rows = np.arange(T)
    i1 = logits.argmax(axis=1)
    tmp = logits.copy()
    tmp[rows, i1] = -np.inf
    i2 = tmp.argmax(axis=1)
    v1 = logits[rows, i1]
    v2 = tmp[rows, i2]
    e2 = np.exp((v2 - v1).astype(np.float32))
    g1 = 1.0 / (1.0 + e2)
    g2 = e2 / (1.0 + e2)
    return i1, i2, g1.astype(np.float32), g2.astype(np.float32)


def _prepare(x, router_w, w1, w2, w3):
    """Route + dispatch on host; returns (C, in_maps, idxs, shape)."""
    x = np.asarray(x, dtype=np.float32)
    router_w = np.asarray(router_w, dtype=np.float32)
    w1 = np.asarray(w1, dtype=np.float32)
    w2 = np.asarray(w2, dtype=np.float32)
    w3 = np.asarray(w3, dtype=np.float32)

    B, S, D = x.shape
    T = B * S
    xf = x.reshape(T, D)

    i1, i2, g1, g2 = _route(xf, router_w)

    # per-expert token lists (slot-0 tokens then slot-1 tokens)
    idxs, wgts = [], []
    for e in range(NUM_EXPERTS):
        s0 = np.nonzero(i1 == e)[0]
        s1 = np.nonzero(i2 == e)[0]
        idxs.append(np.concatenate([s0, s1]))
        wgts.append(np.concatenate([g1[s0], g2[s1]]))
    max_cnt = max(len(ix) for ix in idxs)
    C = max(P, ((max_cnt + P - 1) // P) * P)

    bf16 = ml_dtypes.bfloat16
    # expert weights, padded along the hidden dim and cast to bf16
    w1p = np.zeros((NUM_EXPERTS, D, HP), dtype=bf16)
    w1p[:, :, :HIDDEN] = w1
    w2p = np.zeros((NUM_EXPERTS, D, HP), dtype=bf16)
    w2p[:, :, :HIDDEN] = w2
    w3p = np.zeros((NUM_EXPERTS, HP, D), dtype=bf16)
    w3p[:, :HIDDEN, :] = w3

    in_maps = []
    for e in range(NUM_EXPERTS):
        ix = idxs[e]
        xg = np.zeros((C, D), dtype=np.float32)
        xg[: len(ix)] = xf[ix]
        wvec = np.zeros((C,), dtype=np.float32)
        wvec[: len(ix)] = wgts[e]
        wvec = np.ascontiguousarray(wvec.reshape(C // P, P).T)  # [P, C//P]
        in_maps.append(
            {
                "xt": np.ascontiguousarray(xg.T).astype(bf16).reshape(KD, P, C),
                "w1": w1p[e].reshape(KD, P, HP),
                "w2": w2p[e].reshape(KD, P, HP),
                "w3": w3p[e].reshape(HPT, P, DIM),
                "wv": wvec,
            }
        )
    return C, in_maps, idxs, (B, S, D)


def kernel(x, router_w, w1, w2, w3):
    global LAST_RESULT
    from concourse.bass_utils import run_bass_kernel_spmd

    C, in_maps, idxs, (B, S, D) = _prepare(x, router_w, w1, w2, w3)

    max_cnt = max(len(ix) for ix in idxs)
    key = (C, max_cnt)
    if key not in _KERNELS:
        _KERNELS[key] = _build(C, c_real=max_cnt)
    nc = _KERNELS[key]

    res = run_bass_kernel_spmd(
        nc,
        in_maps,
        list(range(NUM_EXPERTS)),
        trace=TRACE,
    )
    LAST_RESULT = res

    out = np.zeros((B * S, D), dtype=np.float32)
    for e in range(NUM_EXPERTS):
        ix = idxs[e]
        out[ix] += res.results[e]["y"][: len(ix)]
    return out.reshape(B, S, D)
